# revision 25
# baseline (speedup 1.0000x reference)
"""Multi-head causal self-attention (B=4, T=2048, C=1024, H=16) on 8 TRN2 cores.

Sharding: core c handles batch b = c//2 and head-group hg = c%2 (8 heads):
data parallel over B, tensor parallel over H. Each core computes qk^T for its
heads, V in natural layout, causal attention for its 8 heads, and a partial
output projection (row-split W_proj) -> y_partial [T, C]. Host sums
y[b] = y_partial[2b] + y_partial[2b+1] + b_proj.

Key structure (v4):
- q/k pair-packed [chA(64) | chB(64), T]; scores via two concurrent K=64 PE
  row-tiled matmuls (tile_position (0,0)/(64,0)) into adjacent PSUM banks.
- one contiguous ACTIVATE(exp) covers both heads per key-block (head B scores
  are left-aligned to the second bank so diagonal blocks stay contiguous).
- the whole TensorE stream is hand-ordered via explicit add_dep chains:
  2-key-block score platoons (64-row mode) alternate with attnV/projection/
  QKV work (128-row mode), amortizing PE tiling-mode switches, with
  deficit-driven insertion of dense filler work (second-half QKV projection,
  output projection) wherever ScalarE exp would otherwise stall TensorE.
- Z reciprocal per pair runs early (DMA spread/recip/unspread into a
  persistent zeroed rz_all) and the 1/Z broadcast is a full-K=128 matmul so
  it costs no tiling-mode switch.
"""

from contextlib import ExitStack

import ml_dtypes
import numpy as np

import concourse.bass as bass
import concourse.bacc as bacc
import concourse.mybir as mybir
import concourse.tile as tile
from concourse.bass_utils import run_bass_kernel_spmd
from concourse.masks import make_upper_triangular
from concourse.tile_rust import add_dep_helper

B, T, C, H, HS = 4, 2048, 1024, 16, 64
P = 128
NQC = T // 512          # q-chunks of 512
NKB = T // P            # key blocks of 128
TH = T // 2             # t-half
SCALE = HS ** -0.5

F32 = mybir.dt.float32
F32R = mybir.dt.float32r
BF16 = mybir.dt.bfloat16
Exp = mybir.ActivationFunctionType.Exp


def build_kernel():
    nc = bacc.Bacc("TRN2", target_bir_lowering=False)

    # All inputs host-prepacked into the exact SBUF layouts so every DMA is
    # contiguous with >=2KB per-partition lines.
    xt_d = nc.dram_tensor("xt", (C, T), BF16, kind="ExternalInput")
    # wqk: [p, chb, cb, j] -> row p holds all 8 chb-blocks of 8 cb-chunks
    wqk_d = nc.dram_tensor("wqk", (P, 8 * 8 * P), BF16, kind="ExternalInput")
    bqk_d = nc.dram_tensor("bqk", (P, 8), F32, kind="ExternalInput")
    # wv: [p, cb, j]
    wv_d = nc.dram_tensor("wv", (P, 8 * 512), BF16, kind="ExternalInput")
    bv_d = nc.dram_tensor("bv", (1, 512), F32R, kind="ExternalInput")
    # wproj: [r, pr, j]
    wproj_d = nc.dram_tensor("wproj", (P, 4 * C), BF16, kind="ExternalInput")
    y_d = nc.dram_tensor("y", (T, C), F32, kind="ExternalOutput")

    with tile.TileContext(nc) as tc, ExitStack() as big:
        const = big.enter_context(tc.tile_pool(name="const", bufs=1))
        persist = big.enter_context(tc.tile_pool(name="persist", bufs=1))

        # mask[k, q] = 1 where k <= q (valid causal entries of a diag block)
        mask = const.tile([P, P], BF16, tag="mask")
        make_upper_triangular(nc, mask[:], val=1.0, diag=True)
        ones_f = const.tile([P, P], F32, tag="ones_f")
        nc.vector.memset(ones_f[:], 1.0)
        ones_t = const.tile([1, P], F32R, tag="ones")
        nc.vector.tensor_copy(ones_t[:], ones_f[0:1, :])

        # qk_all: 8 blocks of [128, T] bf16; block 2p = q-pair of pair p
        # (rows 0:64 = head 2p channels, rows 64:128 = head 2p+1), block
        # 2p+1 = k-pair with the same row split.
        qk_all = persist.tile([P, 8 * T], BF16, tag="qk")

        # v_all: per (pair, kb): [vA(64) | onesA(1) | vB(64) | onesB(1)] = 130
        v_all = persist.tile([P, 4 * NKB * 130], BF16, tag="v")
        va4 = v_all[:].rearrange("p (a b c) -> p a b c", a=4, b=NKB, c=130)
        nc.vector.tensor_copy(va4[:, :, :, 64:65], ones_f[:, 0 : 4 * NKB])
        nc.vector.tensor_copy(va4[:, :, :, 129:130], ones_f[:, 0 : 4 * NKB])

        # aoT: pair-stacked [128 = ch(head 2p) | ch(head 2p+1), 4 * T] bf16
        aoT = persist.tile([P, 4 * T], BF16, tag="aoT")

        # rz_all: per-pair column block of broadcast-ready 1/Z rows; only
        # rows {32p, 32p+1} of block p are ever non-zero, so the 1/Z
        # broadcast can be a full-K=128 matmul (no PE mode switch).
        rz_all = persist.tile([P, 4 * 512], F32R, tag="rz")
        nc.vector.memset(rz_all[:].bitcast(F32), 0.0)

        # sel2: rows {32p: cols 0:64 = 1}, {32p+1: cols 64:128 = 1}, else 0
        sel2 = const.tile([P, P], F32R, tag="sel2")
        nc.vector.memset(sel2[:].bitcast(F32), 0.0)
        for pr in range(4):
            nc.sync.dma_start(sel2[pr * 32 : pr * 32 + 1, 0:64].bitcast(F32), ones_f[0:1, 0:64])
            nc.sync.dma_start(
                sel2[pr * 32 + 1 : pr * 32 + 2, 64:P].bitcast(F32), ones_f[0:1, 0:64]
            )

        wpj = persist.tile([P, 4 * C], BF16, tag="wpj")

        cp1 = big.enter_context(tc.tile_pool(name="cp1", bufs=1))
        bqk = cp1.tile([P, 8], F32, tag="bqk")
        nc.sync.dma_start(bqk[:], bqk_d[:])
        wv_sb = cp1.tile([P, 8 * 512], BF16, tag="wv")
        bvr = cp1.tile([1, 512], F32R, tag="bvr")
        nc.sync.dma_start(bvr[:], bv_d[:])
        bias_v = cp1.tile([P, 512], F32, tag="bias_v")

        # SBUF pools
        xtp = big.enter_context(tc.tile_pool(name="xtp", bufs=2))
        wp = big.enter_context(tc.tile_pool(name="wp", bufs=2))
        wp1 = big.enter_context(tc.tile_pool(name="wp1", bufs=8))
        atp = big.enter_context(tc.tile_pool(name="atp", bufs=9))
        zrp = big.enter_context(tc.tile_pool(name="zrp", bufs=2))
        zsp_p = big.enter_context(tc.tile_pool(name="zsp_p", bufs=2))
        ysp = big.enter_context(tc.tile_pool(name="ysp", bufs=3))

        # PSUM pools: scores 2x[128,1024] = 4 banks, po0+po1 = 2 banks,
        # generic rotating [128,512] x2 = 2 banks.  Total 8.
        ps_s = big.enter_context(tc.tile_pool(name="ps_s", bufs=2, space="PSUM"))
        ps_o = big.enter_context(tc.tile_pool(name="ps_o", bufs=1, space="PSUM"))
        ps_g = big.enter_context(tc.tile_pool(name="ps_g", bufs=2, space="PSUM"))

        # Emission order doubles as scheduler priority; no hard chaining
        # (the Tile scheduler fills stalls adaptively, which measures faster
        # than a fully hand-ordered TensorE stream).
        def mm(out, lhsT, rhs, **kw):
            return nc.tensor.matmul(out, lhsT, rhs, **kw)

        # PE warmup: keep TensorE busy during the initial DMA wait so the HAM
        # clock gate reaches 8/8 before real matmuls start.  Results unused;
        # the moving operand is uninitialized SBUF (qk_all), which is fine.
        for i in range(26):
            pw = ps_g.tile([P, 512], F32, tag="g", name="pw")
            mm(
                pw[:],
                mask[:],
                qk_all[:, (i % 8) * 512 : (i % 8 + 1) * 512],
                start=True,
                stop=True,
            )

        # bias_v[128, 512] = b_v broadcast along partitions (K=1 matmul)
        pbv = ps_g.tile([P, 512], F32, tag="g")
        mm(pbv[:], ones_t[:], bvr[:], start=True, stop=True)
        nc.vector.tensor_copy(bias_v[:], pbv[:])

        # ---------------- phase-1 emitters (qk^T + natural V) --------------
        def emit_p_qk(th, xT, chb, tcks=(0, 1), wpool=None):
            if wpool is None:
                wb = wp.tile([P, 8 * P], BF16, tag="w", name="wb")
                nc.sync.dma_start(wb[:], wqk_d[:, chb * 1024 : (chb + 1) * 1024])
            else:
                wb = wpool
            for tck in tcks:
                pq = ps_g.tile([P, 512], F32, tag="g", name="pq")
                for cb in range(8):
                    mm(
                        pq[:],
                        wb[:, cb * P : (cb + 1) * P],
                        xT[:, cb * TH + tck * 512 : cb * TH + (tck + 1) * 512],
                        start=(cb == 0),
                        stop=(cb == 7),
                    )
                t0 = th * TH + tck * 512
                nc.vector.tensor_scalar_add(
                    qk_all[:, chb * T + t0 : chb * T + t0 + 512],
                    pq[:],
                    bqk[:, chb : chb + 1],
                )

        def emit_p_v(th, xT, tb):
            kb = th * 8 + tb
            pv = ps_g.tile([P, 512], F32, tag="g", name="pv")
            for cb in range(8):
                mm(
                    pv[:],
                    xT[:, cb * TH + tb * P : cb * TH + (tb + 1) * P],
                    wv_sb[:, cb * 512 : (cb + 1) * 512],
                    start=(cb == 0),
                    stop=(cb == 7),
                )
            dst = bass.AP(
                v_all[:].tensor,
                v_all[:].offset + kb * 130,
                [[v_all[:].ap[0][0], P], [NKB * 130, 4], [65, 2], [1, 64]],
            )
            src = bass.AP(
                pv[:].tensor,
                pv[:].offset,
                [[pv[:].ap[0][0], P], [128, 4], [64, 2], [1, 64]],
            )
            bsrc = bass.AP(
                bias_v[:].tensor,
                bias_v[:].offset,
                [[bias_v[:].ap[0][0], P], [128, 4], [64, 2], [1, 64]],
            )
            nc.vector.tensor_tensor(dst, src, bsrc, mybir.AluOpType.add)

        # ---------------- phase-2 emitters (attention + projection) --------
        def emit_score_platoon(p_pair, qc, kbs):
            """Row-tiled score pairs (64-row mode) + batched exp + mask."""
            qblk, kblk = 2 * p_pair, 2 * p_pair + 1
            q0 = qblk * T + qc * 512
            ats = {}
            for kb in kbs:
                qoff = max(0, kb * P - qc * 512)
                ps2 = ps_s.tile([P, 1024], F32, tag="ps", name="ps2")
                mm(
                    ps2[:, qoff:512],
                    qk_all[0:64, kblk * T + kb * P : kblk * T + (kb + 1) * P],
                    qk_all[0:64, q0 + qoff : q0 + 512],
                    start=True,
                    stop=True,
                )
                mm(
                    ps2[:, 512 : 1024 - qoff],
                    qk_all[64:P, kblk * T + kb * P : kblk * T + (kb + 1) * P],
                    qk_all[64:P, q0 + qoff : q0 + 512],
                    start=True,
                    stop=True,
                )
                at2 = atp.tile([P, 1024], BF16, tag="at", name="at2")
                nc.scalar.activation(
                    at2[:, qoff : 1024 - qoff], ps2[:, qoff : 1024 - qoff],
                    Exp, scale=SCALE,
                )
                if kb * P >= qc * 512:
                    # diagonal block: zero out k > q entries
                    nc.vector.tensor_mul(
                        at2[:, qoff : qoff + P], at2[:, qoff : qoff + P], mask[:]
                    )
                    nc.vector.tensor_mul(
                        at2[:, 512 : 512 + P], at2[:, 512 : 512 + P], mask[:]
                    )
                ats[kb] = (at2, qoff)
            return ats

        def emit_attnv(p_pair, po0, po1, ats, kbs, nkb):
            for kb in kbs:
                at2, qoff = ats[kb]
                base = p_pair * NKB * 130 + kb * 130
                mm(
                    po0[:, qoff:512],
                    v_all[:, base : base + 65],
                    at2[:, qoff:512],
                    start=(kb == 0),
                    stop=(kb == nkb - 1),
                    skip_group_check=True,
                )
                mm(
                    po1[:, qoff:512],
                    v_all[:, base + 65 : base + 130],
                    at2[:, 512 : 1024 - qoff],
                    start=(kb == 0),
                    stop=(kb == nkb - 1),
                    skip_group_check=True,
                )

        def tail_pre(zra, zrb, pr):
            """Z rows -> spread across partitions -> 1/Z -> rz_all block."""
            zsp = zsp_p.tile([P, 32], F32, tag="zsp", name="zsp")
            for hh in range(2):
                r = 2 * pr + hh
                srcz = (zra if hh == 0 else zrb)[pr * 32 : pr * 32 + 1, :]
                nc.sync.dma_start(zsp[r * 16 : (r + 1) * 16, :], srcz)
            zspr = zsp_p.tile([P, 32], F32, tag="zspr", name="zspr")
            nc.vector.reciprocal(
                zspr[pr * 32 : pr * 32 + 32, :], zsp[pr * 32 : pr * 32 + 32, :]
            )
            for hh in range(2):
                r = 2 * pr + hh
                nc.sync.dma_start(
                    rz_all[pr * 32 + hh : pr * 32 + hh + 1, pr * 512 : (pr + 1) * 512].bitcast(F32),
                    zspr[r * 16 : (r + 1) * 16, :],
                )

        def tail_post(qc, pr):
            """Broadcast 1/Z over 128 partitions and scale aoT in place.

            Deliberately NOT in the hand-ordered tensor chain: the broadcast
            matmul is tiny and its 1/Z input arrives via a slow DMA
            spread/unspread pipeline, so the scheduler slots it into a
            natural stall instead of gating the next round's scores.
            """
            col = pr * T + qc * 512
            pbt = ps_g.tile([P, 512], F32, tag="g", name="pbt")
            nc.tensor.matmul(
                pbt[:], sel2[:], rz_all[:, pr * 512 : (pr + 1) * 512],
                start=True, stop=True,
            )
            nc.vector.tensor_mul(
                aoT[0:64, col : col + 512], aoT[0:64, col : col + 512], pbt[0:64, :]
            )
            nc.vector.tensor_mul(
                aoT[64:P, col : col + 512], aoT[64:P, col : col + 512], pbt[64:P, :]
            )

        def emit_proj_group(tb):
            for oc in range(2):
                py = ps_g.tile([P, 512], F32, tag="g", name="py")
                for pp in range(4):
                    mm(
                        py[:],
                        aoT[:, pp * T + tb * P : pp * T + (tb + 1) * P],
                        wpj[:, pp * C + oc * 512 : pp * C + (oc + 1) * 512],
                        start=(pp == 0),
                        stop=(pp == 3),
                    )
                ys = ysp.tile([P, 512], F32, tag="ys", name="ys")
                nc.vector.tensor_copy(ys[:], py[:])
                nc.sync.dma_start(
                    y_d[tb * P : (tb + 1) * P, oc * 512 : (oc + 1) * 512],
                    ys[:],
                )

        # ---------------- emission schedule ----------------
        # Phase 1, first token half.  2 KB-per-partition DMA lines (8 chunks
        # on queues 0-7); the per-chb weight DMAs inside emit_p_qk land on
        # queues 8-15 and are ready almost immediately.
        xT0 = xtp.tile([P, 8 * TH], BF16, tag="xT")
        for cb in range(8):
            for ph in range(2):  # split by partition halves -> 16 queues
                nc.sync.dma_start(
                    xT0[ph * 64 : (ph + 1) * 64, cb * TH : (cb + 1) * TH],
                    xt_d[cb * P + ph * 64 : cb * P + (ph + 1) * 64, 0:TH],
                )
        for chb in range(8):
            emit_p_qk(0, xT0, chb)
        # wv split across 4 queues; needed ~40us in, after the qk groups
        for ph in range(4):
            nc.sync.dma_start(
                wv_sb[ph * 32 : (ph + 1) * 32, :], wv_d[ph * 32 : (ph + 1) * 32, :]
            )
        for tb in range(8):
            emit_p_v(0, xT0, tb)

        # Preload second-half xT + P1 weights + wproj.
        xT1 = xtp.tile([P, 8 * TH], BF16, tag="xT")
        for cb in range(8):
            for ph in range(2):
                nc.sync.dma_start(
                    xT1[ph * 64 : (ph + 1) * 64, cb * TH : (cb + 1) * TH],
                    xt_d[cb * P + ph * 64 : cb * P + (ph + 1) * 64, TH:T],
                )
        wb1 = []
        for chb in range(8):
            wb_t = wp1.tile([P, 8 * P], BF16, tag="w1", name="wb_t")
            nc.sync.dma_start(wb_t[:], wqk_d[:, chb * 1024 : (chb + 1) * 1024])
            wb1.append(wb_t)
        for ph in range(4):
            nc.sync.dma_start(
                wpj[ph * 32 : (ph + 1) * 32, :], wproj_d[ph * 32 : (ph + 1) * 32, :]
            )

        # Filler queues of dense 128-row-mode tensor work, inserted between
        # attention platoons wherever ScalarE exp would stall TensorE.
        # P1a (tokens 1024:1536) must land before qc=2; P1b before qc=3.
        FILLER_EST = 1750.0
        fillers = {0: [], 1: [], 2: [], 3: []}
        for chb in range(6):
            fillers[0].append(
                lambda chb=chb: emit_p_qk(1, xT1, chb, tcks=(0,), wpool=wb1[chb])
            )
        for chb in range(6, 8):
            fillers[1].append(
                lambda chb=chb: emit_p_qk(1, xT1, chb, tcks=(0,), wpool=wb1[chb])
            )
        for tb in range(4):
            fillers[1].append(lambda tb=tb: emit_p_v(1, xT1, tb))
        for chb in range(8):
            fillers[2].append(
                lambda chb=chb: emit_p_qk(1, xT1, chb, tcks=(1,), wpool=wb1[chb])
            )
        for tb in range(4, 8):
            fillers[2].append(lambda tb=tb: emit_p_v(1, xT1, tb))

        state = {"deficit": 0.0}

        def pump(qc, force=False):
            q = fillers.get(qc)
            while q and (force or state["deficit"] >= 0.5 * FILLER_EST):
                q.pop(0)()
                state["deficit"] -= FILLER_EST
            if state["deficit"] < -4000.0:
                state["deficit"] = -4000.0

        def sc_est(qc, kb):
            qoff = max(0, kb * P - qc * 512)
            return 105.0 + (512 - qoff) / 2.4

        def av_est(qc, kb):
            qoff = max(0, kb * P - qc * 512)
            return 120.0 + 2 * (512 - qoff) / 2.4

        def act_est(qc, kb):
            qoff = max(0, kb * P - qc * 512)
            return 110.0 + (1024 - 2 * qoff) / 1.2

        # Deferred-work queue: attnV platoons and pair-tail work pop with a
        # ~2-platoon lag behind the continuous score/exp stream, so the
        # TensorE never waits on a fresh pair's first exp at a boundary.
        LAG = 2
        av_q = []

        def flush_av(n=1):
            for _ in range(n):
                if av_q:
                    av_q.pop(0)()

        def make_av(qc, p_pair, po0, po1, sub, nkb):
            def go():
                emit_attnv(p_pair, po0, po1, sub, sorted(sub), nkb)
                state["deficit"] -= sum(av_est(qc, kb) for kb in sub)
            return go

        def make_pair_tail(qc, p_pair, po0, po1, zra, zrb):
            def go():
                # evict raw ao + Z rows; start this pair's 1/Z pipeline
                col = p_pair * T + qc * 512
                nc.vector.tensor_copy(
                    zra[p_pair * 32 : p_pair * 32 + 1, :], po0[64:65, :]
                )
                nc.vector.tensor_copy(
                    zrb[p_pair * 32 : p_pair * 32 + 1, :], po1[64:65, :]
                )
                nc.vector.tensor_copy(aoT[0:64, col : col + 512], po0[0:64, :])
                nc.vector.tensor_copy(aoT[64:P, col : col + 512], po1[0:64, :])
                tail_pre(zra, zrb, p_pair)
                if qc == NQC - 1 and p_pair >= 1:
                    # last round: normalize pairs as soon as 1/Z is ready
                    tail_post(qc, p_pair - 1)
            return go

        def make_posts(rq):
            def go():
                for pr in range(4):
                    tail_post(rq, pr)
            return go

        pending = None
        for qc in range(NQC):
            if pending is not None:
                # normalize the previous round (pops after its last pair's
                # tail_pre, before this round's rz_all overwrites); its
                # projection becomes filler work for the next round
                av_q.append(make_posts(pending))
                dst = min(qc + 1, 3)
                for tb in range(pending * 4, (pending + 1) * 4):
                    fillers[dst].append(lambda tb=tb: emit_proj_group(tb))
                pending = None
            zra = zrp.tile([P, 512], F32, tag="zra")
            zrb = zrp.tile([P, 512], F32, tag="zrb")
            for p_pair in range(4):
                po0 = ps_o.tile([65, 512], F32, tag="po0")
                po1 = ps_o.tile([65, 512], F32, tag="po1")
                nkb = 4 * qc + 4
                ats = {}
                for kb0 in range(0, nkb, 2):
                    kbs = (kb0, kb0 + 1)
                    ats.update(emit_score_platoon(p_pair, qc, kbs))
                    state["deficit"] += sum(
                        act_est(qc, kb) - sc_est(qc, kb) for kb in kbs
                    )
                    av_q.append(
                        make_av(qc, p_pair, po0, po1, {k: ats[k] for k in kbs}, nkb)
                    )
                    if len(av_q) > LAG:
                        flush_av(len(av_q) - LAG)
                    pump(qc)
                av_q.append(make_pair_tail(qc, p_pair, po0, po1, zra, zrb))
            pump(qc, force=True)
            pending = qc
        flush_av(len(av_q))
        tail_post(3, 3)
        for tb in range(12, 16):
            emit_proj_group(tb)

    nc.compile()
    return nc


def _shard_inputs(x, W_qkv, b_qkv, W_proj):
    """Build the 8 per-core input maps."""
    in_maps = []
    for c in range(8):
        b = c // 2
        hg = c % 2
        heads = [hg * 8 + j for j in range(8)]
        qk_cols = []
        for p in range(4):
            ha, hb = heads[2 * p], heads[2 * p + 1]
            for part in range(2):  # q, k
                qk_cols.extend(range(ha * 192 + part * 64, ha * 192 + part * 64 + 64))
                qk_cols.extend(range(hb * 192 + part * 64, hb * 192 + part * 64 + 64))
        qk_cols = np.array(qk_cols)
        v_cols = []
        for p in range(4):
            ha, hb = heads[2 * p], heads[2 * p + 1]
            v_cols.extend(range(ha * 192 + 128, ha * 192 + 192))
            v_cols.extend(range(hb * 192 + 128, hb * 192 + 192))
        v_cols = np.array(v_cols)
        # Pack weights into the exact SBUF layouts (contiguous 2KB+ DMA lines)
        wqk_p = (
            W_qkv[:, qk_cols]
            .reshape(8, 128, 8, 128)          # [cb, p, chb, j]
            .transpose(1, 2, 0, 3)            # [p, chb, cb, j]
            .reshape(128, 8192)
        )
        wv_p = (
            W_qkv[:, v_cols]
            .reshape(8, 128, 512)             # [cb, p, j]
            .transpose(1, 0, 2)               # [p, cb, j]
            .reshape(128, 4096)
        )
        wpj_p = (
            W_proj[hg * 512 : (hg + 1) * 512, :]
            .reshape(4, 128, 1024)            # [pr, r, j]
            .transpose(1, 0, 2)               # [r, pr, j]
            .reshape(128, 4096)
        )
        in_maps.append(
            {
                "xt": np.ascontiguousarray(x[b].T.astype(ml_dtypes.bfloat16)),
                "wqk": np.ascontiguousarray(wqk_p.astype(ml_dtypes.bfloat16)),
                "bqk": np.ascontiguousarray(
                    b_qkv[qk_cols].reshape(8, 128).T, dtype=np.float32
                ),
                "wv": np.ascontiguousarray(wv_p.astype(ml_dtypes.bfloat16)),
                "bv": np.ascontiguousarray(
                    b_qkv[v_cols].reshape(1, 512), dtype=np.float32
                ),
                "wproj": np.ascontiguousarray(wpj_p.astype(ml_dtypes.bfloat16)),
            }
        )
    return in_maps


_NC = None


def kernel(x, W_qkv, b_qkv, W_proj, b_proj, _trace=False):
    global _NC
    x = np.asarray(x, dtype=np.float32)
    W_qkv = np.asarray(W_qkv, dtype=np.float32)
    b_qkv = np.asarray(b_qkv, dtype=np.float32)
    W_proj = np.asarray(W_proj, dtype=np.float32)
    b_proj = np.asarray(b_proj, dtype=np.float32)

    in_maps = _shard_inputs(x, W_qkv, b_qkv, W_proj)
    if _NC is None:
        _NC = build_kernel()
    res = run_bass_kernel_spmd(
        _NC, in_maps, core_ids=list(range(8)), trace=_trace,
        trace_cores=list(range(8)) if _trace else None,
    )
    out = np.empty((B, T, C), dtype=np.float32)
    for b in range(B):
        out[b] = res.results[2 * b]["y"] + res.results[2 * b + 1]["y"] + b_proj
    if _trace:
        return out, res
    return out


# revision 27
# speedup vs baseline: 1.0016x; 1.0016x over previous
"""Multi-head causal self-attention (B=4, T=2048, C=1024, H=16) on 8 TRN2 cores.

Sharding: core c handles batch b = c//2 and head-group hg = c%2 (8 heads):
data parallel over B, tensor parallel over H. Each core computes qk^T for its
heads, V in natural layout, causal attention for its 8 heads, and a partial
output projection (row-split W_proj) -> y_partial [T, C]. Host sums
y[b] = y_partial[2b] + y_partial[2b+1] + b_proj.

Key structure (v4):
- q/k pair-packed [chA(64) | chB(64), T]; scores via two concurrent K=64 PE
  row-tiled matmuls (tile_position (0,0)/(64,0)) into adjacent PSUM banks.
- one contiguous ACTIVATE(exp) covers both heads per key-block (head B scores
  are left-aligned to the second bank so diagonal blocks stay contiguous).
- the whole TensorE stream is hand-ordered via explicit add_dep chains:
  2-key-block score platoons (64-row mode) alternate with attnV/projection/
  QKV work (128-row mode), amortizing PE tiling-mode switches, with
  deficit-driven insertion of dense filler work (second-half QKV projection,
  output projection) wherever ScalarE exp would otherwise stall TensorE.
- Z reciprocal per pair runs early (DMA spread/recip/unspread into a
  persistent zeroed rz_all) and the 1/Z broadcast is a full-K=128 matmul so
  it costs no tiling-mode switch.
"""

from contextlib import ExitStack

import ml_dtypes
import numpy as np

import concourse.bass as bass
import concourse.bacc as bacc
import concourse.mybir as mybir
import concourse.tile as tile
from concourse.bass_utils import run_bass_kernel_spmd
from concourse.masks import make_upper_triangular
from concourse.tile_rust import add_dep_helper

B, T, C, H, HS = 4, 2048, 1024, 16, 64
P = 128
NQC = T // 512          # q-chunks of 512
NKB = T // P            # key blocks of 128
TH = T // 2             # t-half
SCALE = HS ** -0.5

F32 = mybir.dt.float32
F32R = mybir.dt.float32r
BF16 = mybir.dt.bfloat16
Exp = mybir.ActivationFunctionType.Exp


def build_kernel():
    nc = bacc.Bacc("TRN2", target_bir_lowering=False)

    # All inputs host-prepacked into the exact SBUF layouts so every DMA is
    # contiguous with >=2KB per-partition lines.
    xt_d = nc.dram_tensor("xt", (C, T), BF16, kind="ExternalInput")
    # wqk: [p, chb, cb, j] -> row p holds all 8 chb-blocks of 8 cb-chunks
    wqk_d = nc.dram_tensor("wqk", (P, 8 * 8 * P), BF16, kind="ExternalInput")
    bqk_d = nc.dram_tensor("bqk", (P, 8), F32, kind="ExternalInput")
    # wv: [p, cb, j]
    wv_d = nc.dram_tensor("wv", (P, 8 * 512), BF16, kind="ExternalInput")
    bv_d = nc.dram_tensor("bv", (1, 512), F32R, kind="ExternalInput")
    # wproj: [r, pr, j]
    wproj_d = nc.dram_tensor("wproj", (P, 4 * C), BF16, kind="ExternalInput")
    y_d = nc.dram_tensor("y", (T, C), F32, kind="ExternalOutput")

    with tile.TileContext(nc) as tc, ExitStack() as big:
        const = big.enter_context(tc.tile_pool(name="const", bufs=1))
        persist = big.enter_context(tc.tile_pool(name="persist", bufs=1))

        # mask[k, q] = 1 where k <= q (valid causal entries of a diag block)
        mask = const.tile([P, P], BF16, tag="mask")
        make_upper_triangular(nc, mask[:], val=1.0, diag=True)
        ones_f = const.tile([P, P], F32, tag="ones_f")
        nc.vector.memset(ones_f[:], 1.0)
        ones_t = const.tile([1, P], F32R, tag="ones")
        nc.vector.tensor_copy(ones_t[:], ones_f[0:1, :])

        # qk_all: 8 blocks of [128, T] bf16; block 2p = q-pair of pair p
        # (rows 0:64 = head 2p channels, rows 64:128 = head 2p+1), block
        # 2p+1 = k-pair with the same row split.
        qk_all = persist.tile([P, 8 * T], BF16, tag="qk")

        # v_all: per (pair, kb): [vA(64) | onesA(1) | vB(64) | onesB(1)] = 130
        v_all = persist.tile([P, 4 * NKB * 130], BF16, tag="v")

        # aoT: pair-stacked [128 = ch(head 2p) | ch(head 2p+1), 4 * T] bf16
        aoT = persist.tile([P, 4 * T], BF16, tag="aoT")

        # rz_all: per-pair column block of broadcast-ready 1/Z rows; only
        # rows {32p, 32p+1} of block p are ever non-zero, so the 1/Z
        # broadcast can be a full-K=128 matmul (no PE mode switch).
        rz_all = persist.tile([P, 4 * 512], F32R, tag="rz")

        # sel2: rows {32p: cols 0:64 = 1}, {32p+1: cols 64:128 = 1}, else 0
        sel2 = const.tile([P, P], F32R, tag="sel2")

        wpj = persist.tile([P, 4 * C], BF16, tag="wpj")

        def emit_setup_tail():
            """Setup with DVE dependencies, emitted AFTER the big preload
            DMAs: the Sync sequencer issues DMAs in order, so anything that
            waits on a memset here would otherwise block the xT/weight loads
            behind it in the queue."""
            va4 = v_all[:].rearrange("p (a b c) -> p a b c", a=4, b=NKB, c=130)
            nc.vector.tensor_copy(va4[:, :, :, 64:65], ones_f[:, 0 : 4 * NKB])
            nc.vector.tensor_copy(va4[:, :, :, 129:130], ones_f[:, 0 : 4 * NKB])
            nc.vector.memset(rz_all[:].bitcast(F32), 0.0)
            nc.vector.memset(sel2[:].bitcast(F32), 0.0)
            for pr in range(4):
                nc.sync.dma_start(
                    sel2[pr * 32 : pr * 32 + 1, 0:64].bitcast(F32), ones_f[0:1, 0:64]
                )
                nc.sync.dma_start(
                    sel2[pr * 32 + 1 : pr * 32 + 2, 64:P].bitcast(F32),
                    ones_f[0:1, 0:64],
                )

        cp1 = big.enter_context(tc.tile_pool(name="cp1", bufs=1))
        bqk = cp1.tile([P, 8], F32, tag="bqk")
        nc.sync.dma_start(bqk[:], bqk_d[:])
        wv_sb = cp1.tile([P, 8 * 512], BF16, tag="wv")
        bvr = cp1.tile([1, 512], F32R, tag="bvr")
        nc.sync.dma_start(bvr[:], bv_d[:])
        bias_v = cp1.tile([P, 512], F32, tag="bias_v")

        # SBUF pools
        xtp = big.enter_context(tc.tile_pool(name="xtp", bufs=2))
        wp = big.enter_context(tc.tile_pool(name="wp", bufs=2))
        wp1 = big.enter_context(tc.tile_pool(name="wp1", bufs=8))
        atp = big.enter_context(tc.tile_pool(name="atp", bufs=9))
        zrp = big.enter_context(tc.tile_pool(name="zrp", bufs=2))
        zsp_p = big.enter_context(tc.tile_pool(name="zsp_p", bufs=2))
        ysp = big.enter_context(tc.tile_pool(name="ysp", bufs=3))

        # PSUM pools: scores 2x[128,1024] = 4 banks, po0+po1 = 2 banks,
        # generic rotating [128,512] x2 = 2 banks.  Total 8.
        ps_s = big.enter_context(tc.tile_pool(name="ps_s", bufs=2, space="PSUM"))
        ps_o = big.enter_context(tc.tile_pool(name="ps_o", bufs=1, space="PSUM"))
        ps_g = big.enter_context(tc.tile_pool(name="ps_g", bufs=2, space="PSUM"))

        # Emission order doubles as scheduler priority; no hard chaining
        # (the Tile scheduler fills stalls adaptively, which measures faster
        # than a fully hand-ordered TensorE stream).
        def mm(out, lhsT, rhs, **kw):
            return nc.tensor.matmul(out, lhsT, rhs, **kw)

        # PE warmup: keep TensorE busy during the initial DMA wait so the HAM
        # clock gate reaches 8/8 before real matmuls start.  Results unused;
        # the moving operand is uninitialized SBUF (qk_all), which is fine.
        for i in range(26):
            pw = ps_g.tile([P, 512], F32, tag="g", name="pw")
            mm(
                pw[:],
                mask[:],
                qk_all[:, (i % 8) * 512 : (i % 8 + 1) * 512],
                start=True,
                stop=True,
            )

        # bias_v[128, 512] = b_v broadcast along partitions (K=1 matmul)
        pbv = ps_g.tile([P, 512], F32, tag="g")
        mm(pbv[:], ones_t[:], bvr[:], start=True, stop=True)
        nc.vector.tensor_copy(bias_v[:], pbv[:])

        # ---------------- phase-1 emitters (qk^T + natural V) --------------
        def emit_p_qk(th, xT, chb, tcks=(0, 1), wpool=None):
            if wpool is None:
                wb = wp.tile([P, 8 * P], BF16, tag="w", name="wb")
                nc.sync.dma_start(wb[:], wqk_d[:, chb * 1024 : (chb + 1) * 1024])
            else:
                wb = wpool
            for tck in tcks:
                pq = ps_g.tile([P, 512], F32, tag="g", name="pq")
                for cb in range(8):
                    mm(
                        pq[:],
                        wb[:, cb * P : (cb + 1) * P],
                        xT[:, cb * TH + tck * 512 : cb * TH + (tck + 1) * 512],
                        start=(cb == 0),
                        stop=(cb == 7),
                    )
                t0 = th * TH + tck * 512
                nc.vector.tensor_scalar_add(
                    qk_all[:, chb * T + t0 : chb * T + t0 + 512],
                    pq[:],
                    bqk[:, chb : chb + 1],
                )

        def emit_p_v(th, xT, tb):
            kb = th * 8 + tb
            pv = ps_g.tile([P, 512], F32, tag="g", name="pv")
            for cb in range(8):
                mm(
                    pv[:],
                    xT[:, cb * TH + tb * P : cb * TH + (tb + 1) * P],
                    wv_sb[:, cb * 512 : (cb + 1) * 512],
                    start=(cb == 0),
                    stop=(cb == 7),
                )
            dst = bass.AP(
                v_all[:].tensor,
                v_all[:].offset + kb * 130,
                [[v_all[:].ap[0][0], P], [NKB * 130, 4], [65, 2], [1, 64]],
            )
            src = bass.AP(
                pv[:].tensor,
                pv[:].offset,
                [[pv[:].ap[0][0], P], [128, 4], [64, 2], [1, 64]],
            )
            bsrc = bass.AP(
                bias_v[:].tensor,
                bias_v[:].offset,
                [[bias_v[:].ap[0][0], P], [128, 4], [64, 2], [1, 64]],
            )
            nc.vector.tensor_tensor(dst, src, bsrc, mybir.AluOpType.add)

        # ---------------- phase-2 emitters (attention + projection) --------
        def emit_score_platoon(p_pair, qc, kbs):
            """Row-tiled score pairs (64-row mode) + batched exp + mask."""
            qblk, kblk = 2 * p_pair, 2 * p_pair + 1
            q0 = qblk * T + qc * 512
            ats = {}
            for kb in kbs:
                qoff = max(0, kb * P - qc * 512)
                ps2 = ps_s.tile([P, 1024], F32, tag="ps", name="ps2")
                mm(
                    ps2[:, qoff:512],
                    qk_all[0:64, kblk * T + kb * P : kblk * T + (kb + 1) * P],
                    qk_all[0:64, q0 + qoff : q0 + 512],
                    start=True,
                    stop=True,
                )
                mm(
                    ps2[:, 512 : 1024 - qoff],
                    qk_all[64:P, kblk * T + kb * P : kblk * T + (kb + 1) * P],
                    qk_all[64:P, q0 + qoff : q0 + 512],
                    start=True,
                    stop=True,
                )
                at2 = atp.tile([P, 1024], BF16, tag="at", name="at2")
                nc.scalar.activation(
                    at2[:, qoff : 1024 - qoff], ps2[:, qoff : 1024 - qoff],
                    Exp, scale=SCALE,
                )
                if kb * P >= qc * 512:
                    # diagonal block: zero out k > q entries
                    nc.vector.tensor_mul(
                        at2[:, qoff : qoff + P], at2[:, qoff : qoff + P], mask[:]
                    )
                    nc.vector.tensor_mul(
                        at2[:, 512 : 512 + P], at2[:, 512 : 512 + P], mask[:]
                    )
                ats[kb] = (at2, qoff)
            return ats

        def emit_attnv(p_pair, po0, po1, ats, kbs, nkb):
            for kb in kbs:
                at2, qoff = ats[kb]
                base = p_pair * NKB * 130 + kb * 130
                mm(
                    po0[:, qoff:512],
                    v_all[:, base : base + 65],
                    at2[:, qoff:512],
                    start=(kb == 0),
                    stop=(kb == nkb - 1),
                    skip_group_check=True,
                )
                mm(
                    po1[:, qoff:512],
                    v_all[:, base + 65 : base + 130],
                    at2[:, 512 : 1024 - qoff],
                    start=(kb == 0),
                    stop=(kb == nkb - 1),
                    skip_group_check=True,
                )

        def tail_pre(zra, zrb, pr):
            """Z rows -> spread across partitions -> 1/Z -> rz_all block."""
            zsp = zsp_p.tile([P, 32], F32, tag="zsp", name="zsp")
            for hh in range(2):
                r = 2 * pr + hh
                srcz = (zra if hh == 0 else zrb)[pr * 32 : pr * 32 + 1, :]
                nc.sync.dma_start(zsp[r * 16 : (r + 1) * 16, :], srcz)
            zspr = zsp_p.tile([P, 32], F32, tag="zspr", name="zspr")
            nc.vector.reciprocal(
                zspr[pr * 32 : pr * 32 + 32, :], zsp[pr * 32 : pr * 32 + 32, :]
            )
            for hh in range(2):
                r = 2 * pr + hh
                nc.sync.dma_start(
                    rz_all[pr * 32 + hh : pr * 32 + hh + 1, pr * 512 : (pr + 1) * 512].bitcast(F32),
                    zspr[r * 16 : (r + 1) * 16, :],
                )

        def tail_post(qc, pr):
            """Broadcast 1/Z over 128 partitions and scale aoT in place.

            Deliberately NOT in the hand-ordered tensor chain: the broadcast
            matmul is tiny and its 1/Z input arrives via a slow DMA
            spread/unspread pipeline, so the scheduler slots it into a
            natural stall instead of gating the next round's scores.
            """
            col = pr * T + qc * 512
            pbt = ps_g.tile([P, 512], F32, tag="g", name="pbt")
            nc.tensor.matmul(
                pbt[:], sel2[:], rz_all[:, pr * 512 : (pr + 1) * 512],
                start=True, stop=True,
            )
            nc.vector.tensor_mul(
                aoT[0:64, col : col + 512], aoT[0:64, col : col + 512], pbt[0:64, :]
            )
            nc.vector.tensor_mul(
                aoT[64:P, col : col + 512], aoT[64:P, col : col + 512], pbt[64:P, :]
            )

        def emit_proj_group(tb):
            for oc in range(2):
                py = ps_g.tile([P, 512], F32, tag="g", name="py")
                for pp in range(4):
                    mm(
                        py[:],
                        aoT[:, pp * T + tb * P : pp * T + (tb + 1) * P],
                        wpj[:, pp * C + oc * 512 : pp * C + (oc + 1) * 512],
                        start=(pp == 0),
                        stop=(pp == 3),
                    )
                ys = ysp.tile([P, 512], F32, tag="ys", name="ys")
                nc.vector.tensor_copy(ys[:], py[:])
                nc.sync.dma_start(
                    y_d[tb * P : (tb + 1) * P, oc * 512 : (oc + 1) * 512],
                    ys[:],
                )

        # ---------------- emission schedule ----------------
        # Phase 1, first token half.  2 KB-per-partition DMA lines (8 chunks
        # on queues 0-7); the per-chb weight DMAs inside emit_p_qk land on
        # queues 8-15 and are ready almost immediately.
        xT0 = xtp.tile([P, 8 * TH], BF16, tag="xT")
        for cb in range(8):
            for ph in range(2):  # split by partition halves -> 16 queues
                nc.sync.dma_start(
                    xT0[ph * 64 : (ph + 1) * 64, cb * TH : (cb + 1) * TH],
                    xt_d[cb * P + ph * 64 : cb * P + (ph + 1) * 64, 0:TH],
                )
        for chb in range(8):
            emit_p_qk(0, xT0, chb)
        # wv split across 4 queues; needed ~40us in, after the qk groups
        for ph in range(4):
            nc.sync.dma_start(
                wv_sb[ph * 32 : (ph + 1) * 32, :], wv_d[ph * 32 : (ph + 1) * 32, :]
            )
        for tb in range(8):
            emit_p_v(0, xT0, tb)

        # Preload second-half xT + P1 weights + wproj.
        xT1 = xtp.tile([P, 8 * TH], BF16, tag="xT")
        for cb in range(8):
            for ph in range(2):
                nc.sync.dma_start(
                    xT1[ph * 64 : (ph + 1) * 64, cb * TH : (cb + 1) * TH],
                    xt_d[cb * P + ph * 64 : cb * P + (ph + 1) * 64, TH:T],
                )
        wb1 = []
        for chb in range(8):
            wb_t = wp1.tile([P, 8 * P], BF16, tag="w1", name="wb_t")
            nc.sync.dma_start(wb_t[:], wqk_d[:, chb * 1024 : (chb + 1) * 1024])
            wb1.append(wb_t)
        for ph in range(4):
            nc.sync.dma_start(
                wpj[ph * 32 : (ph + 1) * 32, :], wproj_d[ph * 32 : (ph + 1) * 32, :]
            )
        emit_setup_tail()

        # Filler queues of dense 128-row-mode tensor work, inserted between
        # attention platoons wherever ScalarE exp would stall TensorE.
        # P1a (tokens 1024:1536) must land before qc=2; P1b before qc=3.
        FILLER_EST = 1750.0
        fillers = {0: [], 1: [], 2: [], 3: []}
        for chb in range(6):
            fillers[0].append(
                lambda chb=chb: emit_p_qk(1, xT1, chb, tcks=(0,), wpool=wb1[chb])
            )
        for chb in range(6, 8):
            fillers[1].append(
                lambda chb=chb: emit_p_qk(1, xT1, chb, tcks=(0,), wpool=wb1[chb])
            )
        for tb in range(4):
            fillers[1].append(lambda tb=tb: emit_p_v(1, xT1, tb))
        for chb in range(8):
            fillers[2].append(
                lambda chb=chb: emit_p_qk(1, xT1, chb, tcks=(1,), wpool=wb1[chb])
            )
        for tb in range(4, 8):
            fillers[2].append(lambda tb=tb: emit_p_v(1, xT1, tb))

        state = {"deficit": 0.0}

        def pump(qc, force=False):
            q = fillers.get(qc)
            while q and (force or state["deficit"] >= 0.5 * FILLER_EST):
                q.pop(0)()
                state["deficit"] -= FILLER_EST
            if state["deficit"] < -4000.0:
                state["deficit"] = -4000.0

        def sc_est(qc, kb):
            qoff = max(0, kb * P - qc * 512)
            return 105.0 + (512 - qoff) / 2.4

        def av_est(qc, kb):
            qoff = max(0, kb * P - qc * 512)
            return 120.0 + 2 * (512 - qoff) / 2.4

        def act_est(qc, kb):
            qoff = max(0, kb * P - qc * 512)
            return 110.0 + (1024 - 2 * qoff) / 1.2

        # Deferred-work queue: attnV platoons and pair-tail work pop with a
        # ~2-platoon lag behind the continuous score/exp stream, so the
        # TensorE never waits on a fresh pair's first exp at a boundary.
        LAG = 2
        av_q = []

        def flush_av(n=1):
            for _ in range(n):
                if av_q:
                    av_q.pop(0)()

        def make_av(qc, p_pair, po0, po1, sub, nkb):
            def go():
                emit_attnv(p_pair, po0, po1, sub, sorted(sub), nkb)
                state["deficit"] -= sum(av_est(qc, kb) for kb in sub)
            return go

        def make_pair_tail(qc, p_pair, po0, po1, zra, zrb):
            def go():
                # evict raw ao + Z rows; start this pair's 1/Z pipeline
                col = p_pair * T + qc * 512
                nc.vector.tensor_copy(
                    zra[p_pair * 32 : p_pair * 32 + 1, :], po0[64:65, :]
                )
                nc.vector.tensor_copy(
                    zrb[p_pair * 32 : p_pair * 32 + 1, :], po1[64:65, :]
                )
                nc.vector.tensor_copy(aoT[0:64, col : col + 512], po0[0:64, :])
                nc.vector.tensor_copy(aoT[64:P, col : col + 512], po1[0:64, :])
                tail_pre(zra, zrb, p_pair)
                if qc == NQC - 1 and p_pair >= 1:
                    # last round: normalize pairs as soon as 1/Z is ready
                    tail_post(qc, p_pair - 1)
            return go

        def make_posts(rq):
            def go():
                for pr in range(4):
                    tail_post(rq, pr)
            return go

        pending = None
        for qc in range(NQC):
            if pending is not None:
                # normalize the previous round (pops after its last pair's
                # tail_pre, before this round's rz_all overwrites); its
                # projection becomes filler work for the next round
                av_q.append(make_posts(pending))
                dst = min(qc + 1, 3)
                for tb in range(pending * 4, (pending + 1) * 4):
                    fillers[dst].append(lambda tb=tb: emit_proj_group(tb))
                pending = None
            zra = zrp.tile([P, 512], F32, tag="zra")
            zrb = zrp.tile([P, 512], F32, tag="zrb")
            for p_pair in range(4):
                po0 = ps_o.tile([65, 512], F32, tag="po0")
                po1 = ps_o.tile([65, 512], F32, tag="po1")
                nkb = 4 * qc + 4
                ats = {}
                for kb0 in range(0, nkb, 2):
                    kbs = (kb0, kb0 + 1)
                    ats.update(emit_score_platoon(p_pair, qc, kbs))
                    state["deficit"] += sum(
                        act_est(qc, kb) - sc_est(qc, kb) for kb in kbs
                    )
                    av_q.append(
                        make_av(qc, p_pair, po0, po1, {k: ats[k] for k in kbs}, nkb)
                    )
                    if len(av_q) > LAG:
                        flush_av(len(av_q) - LAG)
                    pump(qc)
                av_q.append(make_pair_tail(qc, p_pair, po0, po1, zra, zrb))
            pump(qc, force=True)
            pending = qc
        flush_av(len(av_q))
        tail_post(3, 3)
        for tb in range(12, 16):
            emit_proj_group(tb)

    nc.compile()
    return nc


def _shard_inputs(x, W_qkv, b_qkv, W_proj):
    """Build the 8 per-core input maps."""
    in_maps = []
    for c in range(8):
        b = c // 2
        hg = c % 2
        heads = [hg * 8 + j for j in range(8)]
        qk_cols = []
        for p in range(4):
            ha, hb = heads[2 * p], heads[2 * p + 1]
            for part in range(2):  # q, k
                qk_cols.extend(range(ha * 192 + part * 64, ha * 192 + part * 64 + 64))
                qk_cols.extend(range(hb * 192 + part * 64, hb * 192 + part * 64 + 64))
        qk_cols = np.array(qk_cols)
        v_cols = []
        for p in range(4):
            ha, hb = heads[2 * p], heads[2 * p + 1]
            v_cols.extend(range(ha * 192 + 128, ha * 192 + 192))
            v_cols.extend(range(hb * 192 + 128, hb * 192 + 192))
        v_cols = np.array(v_cols)
        # Pack weights into the exact SBUF layouts (contiguous 2KB+ DMA lines)
        wqk_p = (
            W_qkv[:, qk_cols]
            .reshape(8, 128, 8, 128)          # [cb, p, chb, j]
            .transpose(1, 2, 0, 3)            # [p, chb, cb, j]
            .reshape(128, 8192)
        )
        wv_p = (
            W_qkv[:, v_cols]
            .reshape(8, 128, 512)             # [cb, p, j]
            .transpose(1, 0, 2)               # [p, cb, j]
            .reshape(128, 4096)
        )
        wpj_p = (
            W_proj[hg * 512 : (hg + 1) * 512, :]
            .reshape(4, 128, 1024)            # [pr, r, j]
            .transpose(1, 0, 2)               # [r, pr, j]
            .reshape(128, 4096)
        )
        in_maps.append(
            {
                "xt": np.ascontiguousarray(x[b].T.astype(ml_dtypes.bfloat16)),
                "wqk": np.ascontiguousarray(wqk_p.astype(ml_dtypes.bfloat16)),
                "bqk": np.ascontiguousarray(
                    b_qkv[qk_cols].reshape(8, 128).T, dtype=np.float32
                ),
                "wv": np.ascontiguousarray(wv_p.astype(ml_dtypes.bfloat16)),
                "bv": np.ascontiguousarray(
                    b_qkv[v_cols].reshape(1, 512), dtype=np.float32
                ),
                "wproj": np.ascontiguousarray(wpj_p.astype(ml_dtypes.bfloat16)),
            }
        )
    return in_maps


_NC = None


def kernel(x, W_qkv, b_qkv, W_proj, b_proj, _trace=False):
    global _NC
    x = np.asarray(x, dtype=np.float32)
    W_qkv = np.asarray(W_qkv, dtype=np.float32)
    b_qkv = np.asarray(b_qkv, dtype=np.float32)
    W_proj = np.asarray(W_proj, dtype=np.float32)
    b_proj = np.asarray(b_proj, dtype=np.float32)

    in_maps = _shard_inputs(x, W_qkv, b_qkv, W_proj)
    if _NC is None:
        _NC = build_kernel()
    res = run_bass_kernel_spmd(
        _NC, in_maps, core_ids=list(range(8)), trace=_trace,
        trace_cores=list(range(8)) if _trace else None,
    )
    out = np.empty((B, T, C), dtype=np.float32)
    for b in range(B):
        out[b] = res.results[2 * b]["y"] + res.results[2 * b + 1]["y"] + b_proj
    if _trace:
        return out, res
    return out


# revision 31
# speedup vs baseline: 1.0181x; 1.0165x over previous
"""Multi-head causal self-attention (B=4, T=2048, C=1024, H=16) on 8 TRN2 cores.

Sharding: core c handles batch b = c//2 and head-group hg = c%2 (8 heads):
data parallel over B, tensor parallel over H. Each core computes qk^T for its
heads, V in natural layout, causal attention for its 8 heads, and a partial
output projection (row-split W_proj) -> y_partial [T, C]. Host sums
y[b] = y_partial[2b] + y_partial[2b+1] + b_proj.

Key structure (v4):
- q/k pair-packed [chA(64) | chB(64), T]; scores via two concurrent K=64 PE
  row-tiled matmuls (tile_position (0,0)/(64,0)) into adjacent PSUM banks.
- one contiguous ACTIVATE(exp) covers both heads per key-block (head B scores
  are left-aligned to the second bank so diagonal blocks stay contiguous).
- the whole TensorE stream is hand-ordered via explicit add_dep chains:
  2-key-block score platoons (64-row mode) alternate with attnV/projection/
  QKV work (128-row mode), amortizing PE tiling-mode switches, with
  deficit-driven insertion of dense filler work (second-half QKV projection,
  output projection) wherever ScalarE exp would otherwise stall TensorE.
- Z reciprocal per pair runs early (DMA spread/recip/unspread into a
  persistent zeroed rz_all) and the 1/Z broadcast is a full-K=128 matmul so
  it costs no tiling-mode switch.
"""

from contextlib import ExitStack

import ml_dtypes
import numpy as np

import concourse.bass as bass
import concourse.bacc as bacc
import concourse.mybir as mybir
import concourse.tile as tile
from concourse.bass_utils import run_bass_kernel_spmd
from concourse.masks import make_upper_triangular
from concourse.tile_rust import add_dep_helper

B, T, C, H, HS = 4, 2048, 1024, 16, 64
P = 128
NQC = T // 512          # q-chunks of 512
NKB = T // P            # key blocks of 128
TH = T // 2             # t-half
SCALE = HS ** -0.5

F32 = mybir.dt.float32
F32R = mybir.dt.float32r
BF16 = mybir.dt.bfloat16
Exp = mybir.ActivationFunctionType.Exp


def build_kernel():
    nc = bacc.Bacc("TRN2", target_bir_lowering=False)

    # All inputs host-prepacked into the exact SBUF layouts so every DMA is
    # contiguous with >=2KB per-partition lines.
    xt_d = nc.dram_tensor("xt", (C, T), BF16, kind="ExternalInput")
    # wqk: [p, chb, cb, j] -> row p holds all 8 chb-blocks of 8 cb-chunks
    wqk_d = nc.dram_tensor("wqk", (P, 8 * 8 * P), BF16, kind="ExternalInput")
    bqk_d = nc.dram_tensor("bqk", (P, 8), F32, kind="ExternalInput")
    # wv: [p, cb, j]
    wv_d = nc.dram_tensor("wv", (P, 8 * 512), BF16, kind="ExternalInput")
    bv_d = nc.dram_tensor("bv", (1, 512), F32R, kind="ExternalInput")
    # wproj: [r, pr, j]
    wproj_d = nc.dram_tensor("wproj", (P, 4 * C), BF16, kind="ExternalInput")
    y_d = nc.dram_tensor("y", (T, C), F32, kind="ExternalOutput")

    with tile.TileContext(nc) as tc, ExitStack() as big:
        const = big.enter_context(tc.tile_pool(name="const", bufs=1))
        persist = big.enter_context(tc.tile_pool(name="persist", bufs=1))

        # mask[k, q] = 1 where k <= q (valid causal entries of a diag block)
        mask = const.tile([P, P], BF16, tag="mask")
        make_upper_triangular(nc, mask[:], val=1.0, diag=True)
        ones_f = const.tile([P, P], F32, tag="ones_f")
        nc.vector.memset(ones_f[:], 1.0)
        ones_t = const.tile([1, P], F32R, tag="ones")
        nc.vector.tensor_copy(ones_t[:], ones_f[0:1, :])

        # qk_all: 8 blocks of [128, T] bf16; block 2p = q-pair of pair p
        # (rows 0:64 = head 2p channels, rows 64:128 = head 2p+1), block
        # 2p+1 = k-pair with the same row split.
        qk_all = persist.tile([P, 8 * T], BF16, tag="qk")

        # v_all: per (pair, kb): [vA(64) | onesA(1) | vB(64) | onesB(1)] = 130
        v_all = persist.tile([P, 4 * NKB * 130], BF16, tag="v")

        # aoT: pair-stacked [128 = ch(head 2p) | ch(head 2p+1), 4 * T] bf16
        aoT = persist.tile([P, 4 * T], BF16, tag="aoT")

        # rz_all: per-pair column block of broadcast-ready 1/Z rows; only
        # rows {32p, 32p+1} of block p are ever non-zero, so the 1/Z
        # broadcast can be a full-K=128 matmul (no PE mode switch).
        rz_all = persist.tile([P, 4 * 512], F32R, tag="rz")

        # sel2: rows {32p: cols 0:64 = 1}, {32p+1: cols 64:128 = 1}, else 0
        sel2 = const.tile([P, P], F32R, tag="sel2")

        wpj = persist.tile([P, 4 * C], BF16, tag="wpj")

        def emit_setup_tail():
            """Setup with DVE dependencies, emitted AFTER the big preload
            DMAs: the Sync sequencer issues DMAs in order, so anything that
            waits on a memset here would otherwise block the xT/weight loads
            behind it in the queue."""
            va4 = v_all[:].rearrange("p (a b c) -> p a b c", a=4, b=NKB, c=130)
            nc.vector.tensor_copy(va4[:, :, :, 64:65], ones_f[:, 0 : 4 * NKB])
            nc.vector.tensor_copy(va4[:, :, :, 129:130], ones_f[:, 0 : 4 * NKB])
            nc.vector.memset(rz_all[:].bitcast(F32), 0.0)
            nc.vector.memset(sel2[:].bitcast(F32), 0.0)
            for pr in range(4):
                nc.sync.dma_start(
                    sel2[pr * 32 : pr * 32 + 1, 0:64].bitcast(F32), ones_f[0:1, 0:64]
                )
                nc.sync.dma_start(
                    sel2[pr * 32 + 1 : pr * 32 + 2, 64:P].bitcast(F32),
                    ones_f[0:1, 0:64],
                )

        cp1 = big.enter_context(tc.tile_pool(name="cp1", bufs=1))
        bqk = cp1.tile([P, 8], F32, tag="bqk")
        nc.sync.dma_start(bqk[:], bqk_d[:])
        wv_sb = cp1.tile([P, 8 * 512], BF16, tag="wv")
        bvr = cp1.tile([1, 512], F32R, tag="bvr")
        nc.sync.dma_start(bvr[:], bv_d[:])
        bias_v = cp1.tile([P, 512], F32, tag="bias_v")

        # SBUF pools
        xtp = big.enter_context(tc.tile_pool(name="xtp", bufs=2))
        wp = big.enter_context(tc.tile_pool(name="wp", bufs=2))
        wp1 = big.enter_context(tc.tile_pool(name="wp1", bufs=8))
        atp = big.enter_context(tc.tile_pool(name="atp", bufs=9))
        zrp = big.enter_context(tc.tile_pool(name="zrp", bufs=2))
        zsp_p = big.enter_context(tc.tile_pool(name="zsp_p", bufs=2))
        ysp = big.enter_context(tc.tile_pool(name="ysp", bufs=3))

        # PSUM pools: scores 2x[128,1024] = 4 banks, po0+po1 = 2 banks,
        # generic rotating [128,512] x2 = 2 banks.  Total 8.
        ps_s = big.enter_context(tc.tile_pool(name="ps_s", bufs=2, space="PSUM"))
        ps_o = big.enter_context(tc.tile_pool(name="ps_o", bufs=1, space="PSUM"))
        ps_g = big.enter_context(tc.tile_pool(name="ps_g", bufs=2, space="PSUM"))

        # Emission order doubles as scheduler priority; no hard chaining
        # (the Tile scheduler fills stalls adaptively, which measures faster
        # than a fully hand-ordered TensorE stream).
        def mm(out, lhsT, rhs, **kw):
            return nc.tensor.matmul(out, lhsT, rhs, **kw)

        # PE warmup: keep TensorE busy during the initial DMA wait so the HAM
        # clock gate reaches 8/8 before real matmuls start.  Results unused;
        # the moving operand is uninitialized SBUF (qk_all), which is fine.
        for i in range(30):
            pw = ps_g.tile([P, 512], F32, tag="g", name="pw")
            mm(
                pw[:],
                mask[:],
                qk_all[:, (i % 8) * 512 : (i % 8 + 1) * 512],
                start=True,
                stop=True,
            )

        # bias_v[128, 512] = b_v broadcast along partitions (K=1 matmul)
        pbv = ps_g.tile([P, 512], F32, tag="g")
        mm(pbv[:], ones_t[:], bvr[:], start=True, stop=True)
        nc.vector.tensor_copy(bias_v[:], pbv[:])

        # ---------------- phase-1 emitters (qk^T + natural V) --------------
        def load_wb(pool, tag, chb):
            wb = pool.tile([P, 8 * P], BF16, tag=tag, name="wb")
            for ph in range(2):
                nc.sync.dma_start(
                    wb[ph * 64 : (ph + 1) * 64, :],
                    wqk_d[ph * 64 : (ph + 1) * 64, chb * 1024 : (chb + 1) * 1024],
                )
            return wb

        def emit_p_qk(th, xT, chb, tcks=(0, 1), wpool=None):
            if wpool is None:
                wb = load_wb(wp, "w", chb)
            else:
                wb = wpool
            for tck in tcks:
                pq = ps_g.tile([P, 512], F32, tag="g", name="pq")
                for cb in range(8):
                    mm(
                        pq[:],
                        wb[:, cb * P : (cb + 1) * P],
                        xT[:, cb * TH + tck * 512 : cb * TH + (tck + 1) * 512],
                        start=(cb == 0),
                        stop=(cb == 7),
                    )
                t0 = th * TH + tck * 512
                nc.vector.tensor_scalar_add(
                    qk_all[:, chb * T + t0 : chb * T + t0 + 512],
                    pq[:],
                    bqk[:, chb : chb + 1],
                )

        def emit_p_v(th, xT, tb):
            kb = th * 8 + tb
            pv = ps_g.tile([P, 512], F32, tag="g", name="pv")
            for cb in range(8):
                mm(
                    pv[:],
                    xT[:, cb * TH + tb * P : cb * TH + (tb + 1) * P],
                    wv_sb[:, cb * 512 : (cb + 1) * 512],
                    start=(cb == 0),
                    stop=(cb == 7),
                )
            dst = bass.AP(
                v_all[:].tensor,
                v_all[:].offset + kb * 130,
                [[v_all[:].ap[0][0], P], [NKB * 130, 4], [65, 2], [1, 64]],
            )
            src = bass.AP(
                pv[:].tensor,
                pv[:].offset,
                [[pv[:].ap[0][0], P], [128, 4], [64, 2], [1, 64]],
            )
            bsrc = bass.AP(
                bias_v[:].tensor,
                bias_v[:].offset,
                [[bias_v[:].ap[0][0], P], [128, 4], [64, 2], [1, 64]],
            )
            nc.vector.tensor_tensor(dst, src, bsrc, mybir.AluOpType.add)

        # ---------------- phase-2 emitters (attention + projection) --------
        def emit_score_platoon(p_pair, qc, kbs):
            """Row-tiled score pairs (64-row mode) + batched exp + mask."""
            qblk, kblk = 2 * p_pair, 2 * p_pair + 1
            q0 = qblk * T + qc * 512
            ats = {}
            for kb in kbs:
                qoff = max(0, kb * P - qc * 512)
                ps2 = ps_s.tile([P, 1024], F32, tag="ps", name="ps2")
                mm(
                    ps2[:, qoff:512],
                    qk_all[0:64, kblk * T + kb * P : kblk * T + (kb + 1) * P],
                    qk_all[0:64, q0 + qoff : q0 + 512],
                    start=True,
                    stop=True,
                )
                mm(
                    ps2[:, 512 : 1024 - qoff],
                    qk_all[64:P, kblk * T + kb * P : kblk * T + (kb + 1) * P],
                    qk_all[64:P, q0 + qoff : q0 + 512],
                    start=True,
                    stop=True,
                )
                at2 = atp.tile([P, 1024], BF16, tag="at", name="at2")
                nc.scalar.activation(
                    at2[:, qoff : 1024 - qoff], ps2[:, qoff : 1024 - qoff],
                    Exp, scale=SCALE,
                )
                if kb * P >= qc * 512:
                    # diagonal block: zero out k > q entries
                    nc.vector.tensor_mul(
                        at2[:, qoff : qoff + P], at2[:, qoff : qoff + P], mask[:]
                    )
                    nc.vector.tensor_mul(
                        at2[:, 512 : 512 + P], at2[:, 512 : 512 + P], mask[:]
                    )
                ats[kb] = (at2, qoff)
            return ats

        def emit_attnv(p_pair, po0, po1, ats, kbs, nkb):
            for kb in kbs:
                at2, qoff = ats[kb]
                base = p_pair * NKB * 130 + kb * 130
                mm(
                    po0[:, qoff:512],
                    v_all[:, base : base + 65],
                    at2[:, qoff:512],
                    start=(kb == 0),
                    stop=(kb == nkb - 1),
                    skip_group_check=True,
                )
                mm(
                    po1[:, qoff:512],
                    v_all[:, base + 65 : base + 130],
                    at2[:, 512 : 1024 - qoff],
                    start=(kb == 0),
                    stop=(kb == nkb - 1),
                    skip_group_check=True,
                )

        def tail_pre(zra, zrb, pr):
            """Z rows -> spread across partitions -> 1/Z -> rz_all block."""
            zsp = zsp_p.tile([P, 32], F32, tag="zsp", name="zsp")
            for hh in range(2):
                r = 2 * pr + hh
                srcz = (zra if hh == 0 else zrb)[pr * 32 : pr * 32 + 1, :]
                nc.sync.dma_start(zsp[r * 16 : (r + 1) * 16, :], srcz)
            zspr = zsp_p.tile([P, 32], F32, tag="zspr", name="zspr")
            nc.vector.reciprocal(
                zspr[pr * 32 : pr * 32 + 32, :], zsp[pr * 32 : pr * 32 + 32, :]
            )
            for hh in range(2):
                r = 2 * pr + hh
                nc.sync.dma_start(
                    rz_all[pr * 32 + hh : pr * 32 + hh + 1, pr * 512 : (pr + 1) * 512].bitcast(F32),
                    zspr[r * 16 : (r + 1) * 16, :],
                )

        def tail_post(qc, pr):
            """Broadcast 1/Z over 128 partitions and scale aoT in place.

            Deliberately NOT in the hand-ordered tensor chain: the broadcast
            matmul is tiny and its 1/Z input arrives via a slow DMA
            spread/unspread pipeline, so the scheduler slots it into a
            natural stall instead of gating the next round's scores.
            """
            col = pr * T + qc * 512
            pbt = ps_g.tile([P, 512], F32, tag="g", name="pbt")
            nc.tensor.matmul(
                pbt[:], sel2[:], rz_all[:, pr * 512 : (pr + 1) * 512],
                start=True, stop=True,
            )
            nc.vector.tensor_mul(
                aoT[0:64, col : col + 512], aoT[0:64, col : col + 512], pbt[0:64, :]
            )
            nc.vector.tensor_mul(
                aoT[64:P, col : col + 512], aoT[64:P, col : col + 512], pbt[64:P, :]
            )

        def emit_proj_group(tb):
            for oc in range(2):
                py = ps_g.tile([P, 512], F32, tag="g", name="py")
                for pp in range(4):
                    mm(
                        py[:],
                        aoT[:, pp * T + tb * P : pp * T + (tb + 1) * P],
                        wpj[:, pp * C + oc * 512 : pp * C + (oc + 1) * 512],
                        start=(pp == 0),
                        stop=(pp == 3),
                    )
                ys = ysp.tile([P, 512], F32, tag="ys", name="ys")
                nc.vector.tensor_copy(ys[:], py[:])
                nc.sync.dma_start(
                    y_d[tb * P : (tb + 1) * P, oc * 512 : (oc + 1) * 512],
                    ys[:],
                )

        # ---------------- emission schedule ----------------
        # Phase 1, first token half.  2 KB-per-partition DMA lines (8 chunks
        # on queues 0-7); the per-chb weight DMAs inside emit_p_qk land on
        # queues 8-15 and are ready almost immediately.
        # wb0 first so it rides idle queues ahead of the xT0 bulk
        wb0 = load_wb(wp, "w", 0)
        xT0 = xtp.tile([P, 8 * TH], BF16, tag="xT")
        for cb in range(8):
            for ph in range(2):  # split by partition halves -> 16 queues
                nc.sync.dma_start(
                    xT0[ph * 64 : (ph + 1) * 64, cb * TH : (cb + 1) * TH],
                    xt_d[cb * P + ph * 64 : cb * P + (ph + 1) * 64, 0:TH],
                )
        emit_p_qk(0, xT0, 0, wpool=wb0)
        for chb in range(1, 8):
            emit_p_qk(0, xT0, chb)
        # wv split across 4 queues; needed ~40us in, after the qk groups
        for ph in range(4):
            nc.sync.dma_start(
                wv_sb[ph * 32 : (ph + 1) * 32, :], wv_d[ph * 32 : (ph + 1) * 32, :]
            )
        for tb in range(8):
            emit_p_v(0, xT0, tb)

        # Preload second-half xT + P1 weights + wproj.
        xT1 = xtp.tile([P, 8 * TH], BF16, tag="xT")
        for cb in range(8):
            for ph in range(2):
                nc.sync.dma_start(
                    xT1[ph * 64 : (ph + 1) * 64, cb * TH : (cb + 1) * TH],
                    xt_d[cb * P + ph * 64 : cb * P + (ph + 1) * 64, TH:T],
                )
        wb1 = [load_wb(wp1, "w1", chb) for chb in range(8)]
        for ph in range(4):
            nc.sync.dma_start(
                wpj[ph * 32 : (ph + 1) * 32, :], wproj_d[ph * 32 : (ph + 1) * 32, :]
            )
        emit_setup_tail()

        # Filler queues of dense 128-row-mode tensor work, inserted between
        # attention platoons wherever ScalarE exp would stall TensorE.
        # P1a (tokens 1024:1536) must land before qc=2; P1b before qc=3.
        FILLER_EST = 1750.0
        fillers = {0: [], 1: [], 2: [], 3: []}
        for chb in range(6):
            fillers[0].append(
                lambda chb=chb: emit_p_qk(1, xT1, chb, tcks=(0,), wpool=wb1[chb])
            )
        for chb in range(6, 8):
            fillers[1].append(
                lambda chb=chb: emit_p_qk(1, xT1, chb, tcks=(0,), wpool=wb1[chb])
            )
        for tb in range(4):
            fillers[1].append(lambda tb=tb: emit_p_v(1, xT1, tb))
        for chb in range(8):
            fillers[2].append(
                lambda chb=chb: emit_p_qk(1, xT1, chb, tcks=(1,), wpool=wb1[chb])
            )
        for tb in range(4, 8):
            fillers[2].append(lambda tb=tb: emit_p_v(1, xT1, tb))

        state = {"deficit": 0.0}

        def pump(qc, force=False):
            q = fillers.get(qc)
            while q and (force or state["deficit"] >= 0.5 * FILLER_EST):
                q.pop(0)()
                state["deficit"] -= FILLER_EST
            if state["deficit"] < -4000.0:
                state["deficit"] = -4000.0

        def sc_est(qc, kb):
            qoff = max(0, kb * P - qc * 512)
            return 105.0 + (512 - qoff) / 2.4

        def av_est(qc, kb):
            qoff = max(0, kb * P - qc * 512)
            return 120.0 + 2 * (512 - qoff) / 2.4

        def act_est(qc, kb):
            qoff = max(0, kb * P - qc * 512)
            return 110.0 + (1024 - 2 * qoff) / 1.2

        # Deferred-work queue: attnV platoons and pair-tail work pop with a
        # ~2-platoon lag behind the continuous score/exp stream, so the
        # TensorE never waits on a fresh pair's first exp at a boundary.
        LAG = 2
        av_q = []

        def flush_av(n=1):
            for _ in range(n):
                if av_q:
                    av_q.pop(0)()

        def make_av(qc, p_pair, po0, po1, sub, nkb):
            def go():
                emit_attnv(p_pair, po0, po1, sub, sorted(sub), nkb)
                state["deficit"] -= sum(av_est(qc, kb) for kb in sub)
            return go

        def make_pair_tail(qc, p_pair, po0, po1, zra, zrb):
            def go():
                # evict raw ao + Z rows; start this pair's 1/Z pipeline
                col = p_pair * T + qc * 512
                nc.vector.tensor_copy(
                    zra[p_pair * 32 : p_pair * 32 + 1, :], po0[64:65, :]
                )
                nc.vector.tensor_copy(
                    zrb[p_pair * 32 : p_pair * 32 + 1, :], po1[64:65, :]
                )
                nc.vector.tensor_copy(aoT[0:64, col : col + 512], po0[0:64, :])
                nc.vector.tensor_copy(aoT[64:P, col : col + 512], po1[0:64, :])
                tail_pre(zra, zrb, p_pair)
                if qc == NQC - 1 and p_pair >= 1:
                    # last round: normalize pairs as soon as 1/Z is ready
                    tail_post(qc, p_pair - 1)
            return go

        def make_posts(rq):
            def go():
                for pr in range(4):
                    tail_post(rq, pr)
            return go

        pending = None
        for qc in range(NQC):
            if pending is not None:
                # normalize the previous round (pops after its last pair's
                # tail_pre, before this round's rz_all overwrites); its
                # projection becomes filler work for the next round
                av_q.append(make_posts(pending))
                dst = min(qc + 1, 3)
                for tb in range(pending * 4, (pending + 1) * 4):
                    fillers[dst].append(lambda tb=tb: emit_proj_group(tb))
                pending = None
            zra = zrp.tile([P, 512], F32, tag="zra")
            zrb = zrp.tile([P, 512], F32, tag="zrb")
            for p_pair in range(4):
                po0 = ps_o.tile([65, 512], F32, tag="po0")
                po1 = ps_o.tile([65, 512], F32, tag="po1")
                nkb = 4 * qc + 4
                ats = {}
                for kb0 in range(0, nkb, 2):
                    kbs = (kb0, kb0 + 1)
                    ats.update(emit_score_platoon(p_pair, qc, kbs))
                    state["deficit"] += sum(
                        act_est(qc, kb) - sc_est(qc, kb) for kb in kbs
                    )
                    av_q.append(
                        make_av(qc, p_pair, po0, po1, {k: ats[k] for k in kbs}, nkb)
                    )
                    if len(av_q) > LAG:
                        flush_av(len(av_q) - LAG)
                    pump(qc)
                av_q.append(make_pair_tail(qc, p_pair, po0, po1, zra, zrb))
            pump(qc, force=True)
            pending = qc
        flush_av(len(av_q))
        tail_post(3, 3)
        for tb in range(12, 16):
            emit_proj_group(tb)

    nc.compile()
    return nc


def _shard_inputs(x, W_qkv, b_qkv, W_proj):
    """Build the 8 per-core input maps."""
    in_maps = []
    for c in range(8):
        b = c // 2
        hg = c % 2
        heads = [hg * 8 + j for j in range(8)]
        qk_cols = []
        for p in range(4):
            ha, hb = heads[2 * p], heads[2 * p + 1]
            for part in range(2):  # q, k
                qk_cols.extend(range(ha * 192 + part * 64, ha * 192 + part * 64 + 64))
                qk_cols.extend(range(hb * 192 + part * 64, hb * 192 + part * 64 + 64))
        qk_cols = np.array(qk_cols)
        v_cols = []
        for p in range(4):
            ha, hb = heads[2 * p], heads[2 * p + 1]
            v_cols.extend(range(ha * 192 + 128, ha * 192 + 192))
            v_cols.extend(range(hb * 192 + 128, hb * 192 + 192))
        v_cols = np.array(v_cols)
        # Pack weights into the exact SBUF layouts (contiguous 2KB+ DMA lines)
        wqk_p = (
            W_qkv[:, qk_cols]
            .reshape(8, 128, 8, 128)          # [cb, p, chb, j]
            .transpose(1, 2, 0, 3)            # [p, chb, cb, j]
            .reshape(128, 8192)
        )
        wv_p = (
            W_qkv[:, v_cols]
            .reshape(8, 128, 512)             # [cb, p, j]
            .transpose(1, 0, 2)               # [p, cb, j]
            .reshape(128, 4096)
        )
        wpj_p = (
            W_proj[hg * 512 : (hg + 1) * 512, :]
            .reshape(4, 128, 1024)            # [pr, r, j]
            .transpose(1, 0, 2)               # [r, pr, j]
            .reshape(128, 4096)
        )
        in_maps.append(
            {
                "xt": np.ascontiguousarray(x[b].T.astype(ml_dtypes.bfloat16)),
                "wqk": np.ascontiguousarray(wqk_p.astype(ml_dtypes.bfloat16)),
                "bqk": np.ascontiguousarray(
                    b_qkv[qk_cols].reshape(8, 128).T, dtype=np.float32
                ),
                "wv": np.ascontiguousarray(wv_p.astype(ml_dtypes.bfloat16)),
                "bv": np.ascontiguousarray(
                    b_qkv[v_cols].reshape(1, 512), dtype=np.float32
                ),
                "wproj": np.ascontiguousarray(wpj_p.astype(ml_dtypes.bfloat16)),
            }
        )
    return in_maps


_NC = None


def kernel(x, W_qkv, b_qkv, W_proj, b_proj, _trace=False):
    global _NC
    x = np.asarray(x, dtype=np.float32)
    W_qkv = np.asarray(W_qkv, dtype=np.float32)
    b_qkv = np.asarray(b_qkv, dtype=np.float32)
    W_proj = np.asarray(W_proj, dtype=np.float32)
    b_proj = np.asarray(b_proj, dtype=np.float32)

    in_maps = _shard_inputs(x, W_qkv, b_qkv, W_proj)
    if _NC is None:
        _NC = build_kernel()
    res = run_bass_kernel_spmd(
        _NC, in_maps, core_ids=list(range(8)), trace=_trace,
        trace_cores=list(range(8)) if _trace else None,
    )
    out = np.empty((B, T, C), dtype=np.float32)
    for b in range(B):
        out[b] = res.results[2 * b]["y"] + res.results[2 * b + 1]["y"] + b_proj
    if _trace:
        return out, res
    return out
